# revision 75
# baseline (speedup 1.0000x reference)
"""Causal single-head attention (B=4, T=2048, E=1024, H=128) on 8 NeuronCores.

Sharding: core = (batch b, parity h); block-cyclic q-blocks {h, h+2, ..} (stage
s owns global q-block 2s+h; each key-slot pair stores the own block first so
the SPMD program is identical across cores).

Projections run as fp8e4(DoubleRow) with two-term error compensation:
x = xh + xl and W*32 = Wh + Wl (both fp8); psum accumulates the three passes
xh*Wh + xl*Wh + xh*Wl (xl*Wl ~ 0.4% of one quantum, dropped). DoubleRow pairs
e-chunks, contracting 256 rows per matmul at 0.5 cycles/output-row, so the
projection block costs 3/8 of fp16 at ~2e-3 end-to-end error. Scores and AV
stay fp16. W is pre-scaled by 32 on the host (fp8 normals); kqT holds
32*(k+bk) / 32*(q+bq) in fp16 and exp() applies scale 1/(1024*sqrt(H)) to the
fp32 psum scores directly. Causality is a 0/1 fp16 multiply on the exp output
(tri for the diagonal block; all-0/1 per-core broadcast for the pair slot).
Softmax denominators ride AV as a ones column in vaug; the division (and
+bv) happens on the host.

Schedule: a dummy-matmul block covers the initial DMA wait and pre-ramps the
PE p-state. x is packed block-major ([s][j][hl][e][pos]) so stage 7's OWN
block can be DMA'd first and its Q projection emitted immediately - that
unblocks the big stage's score chunks as soon as each K/V pair lands,
spreading the ACT exp stream uniformly instead of bunching it at the tail.
Score chunks (4 key slots) are emitted eagerly at the emission slot where
their K pairs exist; exp results wait in SBUF ex tiles so each stage's
AV+output runs with no exp latency on the critical path. DMAs: weights
(hi,lo) + x halves on the sync HWDGE queue, y outputs on sync (last two on
scalar/sync), consts via gpsimd SWDGE.
"""

import math

import numpy as np

import concourse.tile as tile
from concourse import bacc, mybir
from concourse.bass_utils import run_bass_kernel_spmd

B, T, E, H = 4, 2048, 1024, 128
NB = T // 128        # 16 key slots
NE = E // 128        # 8 contraction chunks
NS = 8               # stages (local q-blocks) per core

F32 = mybir.dt.float32
F16 = mybir.dt.float16
F8 = mybir.dt.float8e4
DR = mybir.MatmulPerfMode.DoubleRow

EXPSC = float(1.0 / (1024.0 * math.sqrt(H)))
NDUMMY = 21          # PE warm-up matmuls covering the head DMA wait

_CACHE: dict = {}


def _chunks(s):
    n = 2 * s + 2
    if s >= 5:
        # big stages lead with a 2-slot chunk: it only needs K pair 0, so
        # its scores fill the PE wait for the x DMA that feeds KV pair 1
        return [(0, 2)] + [(c0, min(c0 + 4, n)) for c0 in range(2, n, 4)]
    return [(c0, min(c0 + 4, n)) for c0 in range(0, n, 4)]


def _valid_slot(s, c):
    # emission slot (after proj-pair k) where chunk c of stage s has its K
    # pairs and its q; stages 5-7 have their q projected early (their own
    # x block is DMA'd ahead), so only the K-pair need gates them
    c0, c1 = _chunks(s)[c]
    pair_need = (c1 - 1) >> 1
    if s == NS - 1:
        return max(pair_need, 0.5)   # q7 emitted right after slot 0
    if s > 4:
        # q5/q6 are emitted right after slot 3 (their own x lands then);
        # their pair-0-only lead chunks follow at bucket 3.5
        return max(pair_need, 3.5)
    return max(pair_need, s)


def _build():
    nc = bacc.Bacc(None, target_bir_lowering=False)
    # x: [s][j(own/oth)][hl][e][pos], 4KB/partition per stage
    xd = nc.dram_tensor("xd", [128, NS * 4096], F8, kind="ExternalInput")
    wd = nc.dram_tensor("wd", [128, 2 * 3 * NE * 128], F8, kind="ExternalInput")
    bkd = nc.dram_tensor("bkd", [128, 1], F32, kind="ExternalInput")
    bqd = nc.dram_tensor("bqd", [128, 1], F32, kind="ExternalInput")
    bxd = nc.dram_tensor("bxd", [128, 1], F32, kind="ExternalInput")
    # y is partition-major: y[q, s*129 + c] (stage s, query row q)
    y = nc.dram_tensor("y", [128, NS * 129], F32, kind="ExternalOutput")

    with tile.TileContext(nc) as tc:
        with (
            tc.tile_pool(name="xs", bufs=1) as xs_pool,
            tc.tile_pool(name="ws", bufs=1) as w_pool,
            tc.tile_pool(name="small", bufs=1) as sm_pool,
            tc.tile_pool(name="qk", bufs=1) as qk_pool,
            tc.tile_pool(name="ex", bufs=20) as ex_pool,
            tc.tile_pool(name="ob", bufs=4) as ob_pool,
            tc.tile_pool(name="pp", bufs=3, space="PSUM") as pp_pool,
            tc.tile_pool(name="sps", bufs=3, space="PSUM") as sps_pool,
            tc.tile_pool(name="avp", bufs=2, space="PSUM") as av_pool,
        ):
            xs = xs_pool.tile([128, NS, 2, 2, NE, 128], F8)
            ws = w_pool.tile([128, 2, 3, NE, 128], F8)
            bk_sb = sm_pool.tile([128, 1], F32, tag="bk")
            bq_sb = sm_pool.tile([128, 1], F32, tag="bq")
            bx_sb = sm_pool.tile([128, 1], F32, tag="bx")
            zb = sm_pool.tile([128, 1], F32, tag="zb")
            mask_sb = sm_pool.tile([128, 256], F16, tag="msk")
            dum = sm_pool.tile([128, 256], F16, tag="dum")
            # kqT: per stage [k pair 256 | q 128] fp16, all *32 (+bias)
            kqT = qk_pool.tile([128, NS * 384], F16, tag="kqT")
            vaug = qk_pool.tile([128, NB * 129], F16, tag="vaug")

            def kslot(kb):
                p = (kb >> 1) * 384 + (kb & 1) * 128
                return kqT[:, p:p + 128]

            def qslot(s):
                return kqT[:, s * 384 + 256:s * 384 + 384]

            # PE warm-up: matmuls on a zeroed tile cover the first x/w DMAs
            # and bring the p-state to full speed before real work arrives
            nc.gpsimd.memset(dum[:], 0.0)
            pdum = sps_pool.tile([128, 512], F32, tag="sp", name="pdum")
            for _ in range(NDUMMY):
                nc.tensor.matmul(pdum[:, 0:256], dum[:, 0:128], dum[:],
                                 start=True, stop=True)

            # consts + mask build off the critical DMA path (SWDGE / Pool)
            nc.gpsimd.dma_start(bk_sb[:], bkd[:])
            nc.gpsimd.dma_start(bq_sb[:], bqd[:])
            nc.gpsimd.dma_start(bx_sb[:], bxd[:])
            nc.gpsimd.memset(zb[:], 0.0)
            nc.gpsimd.memset(mask_sb[:], 1.0)
            nc.gpsimd.affine_select(
                out=mask_sb[:, 0:128], in_=mask_sb[:, 0:128],
                compare_op=mybir.AluOpType.is_ge, fill=0.0,
                base=0, pattern=[[1, 128]], channel_multiplier=-1,
            )  # keep 1 where key p <= query c, else 0
            nc.gpsimd.tensor_scalar_mul(
                mask_sb[:, 128:256], mask_sb[:, 128:256], bx_sb[:])
            # ones columns for the softmax denominators (strided, 16 cols)
            nc.gpsimd.memset(vaug[:, 128::129], 1.0)

            # DMA order: big stages' own blocks early (unblocks q7/q6/q5 and
            # with them the eager score chunks); oth halves pace the KVs
            def dx(s, j):
                return nc.sync.dma_start(
                    xs[:, s, j], xd[:, s * 4096 + j * 2048:
                                    s * 4096 + (j + 1) * 2048])
            nc.sync.dma_start(ws[:, 0], wd[:, 0:3072])
            nc.sync.dma_start(ws[:, 1], wd[:, 3072:6144])
            dx(0, 0)
            dx(0, 1)
            dx(7, 0)
            dx(1, 0)
            dx(1, 1)
            for s in range(2, 4):
                dx(s, 0)
                dx(s, 1)
            dx(6, 0)
            dx(5, 0)
            dx(4, 0)
            dx(4, 1)
            dx(5, 1)
            dx(6, 1)
            dx(7, 1)

            # (x half, w half): hi*hi + lo*hi + hi*lo = x@W up to ~2e-3
            PASSES = ((0, 0), (1, 0), (0, 1))

            def emit_q(s, pq):
                for pi, (xh_, wh_) in enumerate(PASSES):
                    for e in range(0, NE, 2):
                        nc.tensor.matmul(
                            pq, ws[:, wh_, 1, e:e + 2, :],
                            xs[:, s, 0, xh_, e:e + 2, :],
                            start=(pi == 0 and e == 0),
                            stop=(pi == len(PASSES) - 1 and e == NE - 2),
                            perf_mode=DR)

            def emit_kv(s, pk, pv):
                for j in range(2):
                    for pi, (xh_, wh_) in enumerate(PASSES):
                        for e in range(0, NE, 2):
                            nc.tensor.matmul(
                                pk[:, j * 128:(j + 1) * 128],
                                ws[:, wh_, 0, e:e + 2, :],
                                xs[:, s, j, xh_, e:e + 2, :],
                                start=(pi == 0 and e == 0),
                                stop=(pi == len(PASSES) - 1 and e == NE - 2),
                                perf_mode=DR)
                for j in range(2):
                    for pi, (xh_, wh_) in enumerate(PASSES):
                        for e in range(0, NE, 2):
                            nc.tensor.matmul(
                                pv[:, j * 128:(j + 1) * 128],
                                xs[:, s, j, xh_, e:e + 2, :],
                                ws[:, wh_, 2, e:e + 2, :],
                                start=(pi == 0 and e == 0),
                                stop=(pi == len(PASSES) - 1 and e == NE - 2),
                                perf_mode=DR)

            def post_k(s, pk):
                nc.vector.tensor_scalar_add(
                    kqT[:, s * 384:s * 384 + 256], pk, bk_sb[:])

            def post_q(s, pq):
                nc.vector.tensor_scalar_add(qslot(s), pq, bq_sb[:])

            def post_v(s, pv):
                for j in range(2):
                    kb = 2 * s + j
                    nc.vector.tensor_copy(
                        vaug[:, kb * 129:kb * 129 + 128],
                        pv[:, j * 128:(j + 1) * 128])

            exs = {}     # (s, c) -> ex tile

            def emit_chunk(s, c):
                c0, c1 = _chunks(s)[c]
                n = 2 * s + 2
                w = (c1 - c0) * 128
                sp = sps_pool.tile([128, 512], F32, tag="sp")
                for kb in range(c0, c1):
                    m = kb - c0
                    nc.tensor.matmul(
                        sp[:, m * 128:(m + 1) * 128], kslot(kb), qslot(s),
                        start=True, stop=True)
                ex = ex_pool.tile([128, 512], F16)
                exs[(s, c)] = ex
                nc.scalar.activation(
                    ex[:, 0:w], sp[:, 0:w],
                    mybir.ActivationFunctionType.Exp,
                    bias=zb[:], scale=EXPSC)
                if c1 == n:  # causal masks live on the last two slots
                    nc.vector.tensor_tensor(
                        ex[:, w - 256:w], ex[:, w - 256:w], mask_sb[:],
                        mybir.AluOpType.mult)

            def emit_av(s, av=None, chunk_lo=0, chunk_hi=None, cont=False,
                        stop=True):
                # accumulate AV over chunks [chunk_lo, chunk_hi); cont=True
                # continues an open accumulation (no start), stop=False
                # leaves it open for a later continuation
                chunks = _chunks(s)
                if chunk_hi is None:
                    chunk_hi = len(chunks)
                if av is None:
                    av_t = av_pool.tile([128, 256], F32, tag="av", name="av")
                    av = av_t[:, 0:129]
                first = chunks[chunk_lo][0]
                last = chunks[chunk_hi - 1][1] - 1
                for c in range(chunk_lo, chunk_hi):
                    c0, c1 = chunks[c]
                    for kb in range(c0, c1):
                        m = kb - c0
                        nc.tensor.matmul(
                            av, exs[(s, c)][:, m * 128:(m + 1) * 128],
                            vaug[:, kb * 129:(kb + 1) * 129],
                            start=(kb == first and not cont),
                            stop=(kb == last and stop))
                return av

            sched = {}
            for s in range(NS):
                for c in range(len(_chunks(s))):
                    sched.setdefault(_valid_slot(s, c), []).append((s, c))

            # q solos: q7 right after slot 0, q6/q5 after slot 1 (as their
            # own x halves land)
            def emit_q_solo(s):
                pqt = pp_pool.tile([128, 512], F32, tag="pp", name="pqt")
                emit_q(s, pqt[:, 256:384])
                post_q(s, pqt[:, 256:384])

            def emit_slot(k):
                pp = pp_pool.tile([128, 512], F32, tag="pp", name="pp")
                pk, pq = pp[:, 0:256], pp[:, 256:384]
                pv = av_pool.tile([128, 256], F32, tag="av", name="pv")
                emit_kv(k, pk, pv)
                if k <= 4:
                    emit_q(k, pq)
                    post_q(k, pq)
                post_k(k, pk)
                post_v(k, pv)
                def do_av():
                    if k >= 2:
                        # AV of stage k-2 is PE work with no fresh deps: it
                        # covers the DVE post latency before the new scores
                        s_out = k - 2
                        av = emit_av(s_out)
                        ob = ob_pool.tile([128, 129], F32, tag="ob")
                        nc.vector.tensor_copy(ob[:], av)
                        nc.sync.dma_start(
                            y[:, s_out * 129:(s_out + 1) * 129], ob[:])

                def do_chunks():
                    for (s, c) in sched.get(k, []):
                        emit_chunk(s, c)

                do_av()
                do_chunks()

            def out_dma(av, col, eng):
                ob = ob_pool.tile([128, 129], F32, tag="ob")
                nc.vector.tensor_copy(ob[:], av)
                eng.dma_start(y[:, col * 129:(col + 1) * 129], ob[:])

            emit_slot(0)
            emit_q_solo(NS - 1)
            for (s, c) in sched.get(0.5, []):
                emit_chunk(s, c)
            emit_slot(1)
            emit_slot(2)
            emit_slot(3)
            emit_q_solo(NS - 2)
            emit_q_solo(NS - 3)
            for (s, c) in sched.get(3.5, []):
                emit_chunk(s, c)
            for k in range(4, NS):
                emit_slot(k)
            # tail: only the two biggest stages remain (AV5 rode slot 7)
            out_dma(emit_av(6), 6, nc.scalar)
            out_dma(emit_av(7), 7, nc.sync)
    nc.compile()
    return nc


def _pack_w(w: np.ndarray) -> np.ndarray:
    # [E, H] -> [128, NE, H]: chunk e, partitions = rows e*128+p
    return np.ascontiguousarray(w.reshape(NE, 128, H).transpose(1, 0, 2))


def kernel(x, Wq, bq, Wk, bk, Wv, bv):
    import ml_dtypes
    F8NP = ml_dtypes.float8_e4m3

    x = np.asarray(x, dtype=np.float32)
    Wq = np.asarray(Wq, dtype=np.float32)
    Wk = np.asarray(Wk, dtype=np.float32)
    Wv = np.asarray(Wv, dtype=np.float32)
    bq = np.asarray(bq, dtype=np.float32)
    bk = np.asarray(bk, dtype=np.float32)
    bv = np.asarray(bv, dtype=np.float32)

    if "nc" not in _CACHE:
        _CACHE["nc"] = _build()
    nc = _CACHE["nc"]

    # weights: *32 so fp8 stays in normals; hi/lo error compensation
    whl = []
    for W in (Wk, Wq, Wv):
        W32 = W * np.float32(32.0)
        hi = W32.astype(F8NP)
        lo = (W32 - hi.astype(np.float32)).astype(F8NP)
        whl.append((_pack_w(hi.astype(np.float32)), _pack_w(lo.astype(np.float32))))
    wsb = np.empty((128, 2, 3, NE, H), dtype=F8NP)
    for k in range(3):
        wsb[:, 0, k] = whl[k][0]
        wsb[:, 1, k] = whl[k][1]
    wsb = np.ascontiguousarray(wsb.reshape(128, -1))

    bk_s = np.ascontiguousarray((bk * 32.0).reshape(H, 1)).astype(np.float32)
    bq_s = np.ascontiguousarray((bq * 32.0).reshape(H, 1)).astype(np.float32)
    bxs = {0: np.zeros((128, 1), np.float32), 1: np.ones((128, 1), np.float32)}

    xh = x.astype(F8NP)
    xl = (x - xh.astype(np.float32)).astype(F8NP)
    # [b, blk, pos, e, ep]
    xhb = xh.reshape(B, NB, 128, NE, 128)
    xlb = xl.reshape(B, NB, 128, NE, 128)

    def pack_core(xb, b, h):
        own = xb[b, h::2]          # [s, pos, e, ep]
        oth = xb[b, 1 - h::2]
        pair = np.stack([own, oth], axis=1)            # [s, j, pos, e, ep]
        return pair.transpose(4, 0, 1, 3, 2)           # [ep, s, j, e, pos]

    in_maps = []
    for core in range(8):
        b, h = divmod(core, 2)
        hi = pack_core(xhb, b, h)
        lo = pack_core(xlb, b, h)
        # [128, s, j, hl, e, pos]
        xsc = np.ascontiguousarray(
            np.stack([hi, lo], axis=3).reshape(128, -1))
        in_maps.append({
            "xd": xsc, "wd": wsb, "bkd": bk_s, "bqd": bq_s, "bxd": bxs[h],
        })

    res = run_bass_kernel_spmd(nc, in_maps, core_ids=list(range(8)))
    out = np.empty((B, T, H), dtype=np.float32)
    for core in range(8):
        b, h = divmod(core, 2)
        yc = res.results[core]["y"].reshape(128, NS, 129)
        num = yc[:, :, 0:128]                      # [q, s, H]
        den = yc[:, :, 128:129]
        blocks = (num / den / np.float32(32.0)).transpose(1, 0, 2)
        for s in range(NS):
            g = 2 * s + h
            out[b, g * 128:(g + 1) * 128, :] = blocks[s]
    out += bv  # softmax rows sum to 1, so +bv commutes with attention
    return out
